# revision 1
# baseline (speedup 1.0000x reference)
"""BitConv2d (BitNet-style fake-quant 3x3 conv) Bass/Tile kernel for TRN2.

Data-parallel over batch: 16 images -> 8 NeuronCores x 2 images. The global
absmax activation scale is computed on-device with an AllReduce(max) so
quantization matches single-device semantics.

Math (matches the reference nn.Module):
  x_scale = max(|x|) + 1e-5            (global over the full batch)
  x_q = round(clip(x*127/x_scale))     (round-to-nearest-even via magic add;
                                        clip never binds since |x*127/s| < 127)
  w_scale = mean(|w|) + 1e-5
  w_q = clip(round(w/w_scale), -1, 1)
  out = conv3x3_pad1(x_q, w_q) * (x_scale/127) * w_scale
x_q and w_q are small integers, exactly representable in bf16; their conv
accumulates exactly in fp32 PSUM, so the only rounding vs the fp32 reference
is in the final scale multiplies (~1e-7 relative) plus rare one-LSB round
flips from computing x*(127/s) instead of (x*127)/s.

Per-core layout (n_img images of [32, H, W], strip = HS = H/4 rows):
  SBUF partition p = 4*c + s  (c = in-channel, s = strip index 0..3).
  With this permutation the DRAM address of partition p's strip is LINEAR in
  p (stride PR = HS*W elements) for x (p = 4c+s) and out (m = 4o+s), so all
  transfers are pure-2D DMAs [uniform partition stride, contiguous run] -
  anything with more AP dims runs several times slower on the DGE, and
  strided-partition SBUF APs break Tile's dependency tracking entirely.

  x is DMA'd into CONTIGUOUS per-chunk tiles; the quantize pass writes x_q
  into a PADDED per-image buffer: BROWS = HS+3 rows x PW = W+2 cols:
    row 0 = top halo (s>0) / zero pad (s=0); rows 1..HS = strip rows with
    zero pad cols 0, W+1; row HS+1 = bottom halo (s<3) / zero pad; +1 slack.
  A conv tap (dy,dx) is then a free-dim offset dy*PW+dx: output tile k
  (2 padded rows) reads x_q[:, 2PW*k + PW*dy + dx : +2PW].
  Matmul: lhsT[p=(4c+s), m=(4o+s)] = w_q[o,c,dy,dx] (block-diagonal over
  strips), K=128, M=128, N=2PW (one PSUM bank), accumulating the 9 taps.
  Drain reads PSUM with the padded pitch into contiguous staging; one 2D
  DMA per 8-row super-tile stores it.

Halo rows: the halo address is 12544p - 224 (top) / +12544 (bottom), linear
in p, so a single 2D DMA over partitions 1..127 (resp. 0..126) fetches all
of them; the pad strips receive neighboring-channel garbage which is zeroed
by quantizing halos with a per-partition MASKED scale (p%4==0 / ==3 -> 0).

The block-diagonal lhsT is built on the PE: psum[p,m] = wq[o(m), c(p)] via
(spread-matrix).T @ (wq columns repeated 4x with a stride-0 AP), then a
constant mod-4 block mask is applied on the way to SBUF. The spread matrix
and mask come from iota/affine_select, so nothing uses strided partitions.

Engine budget: PE runs the matmuls (~100us dense). DVE: absmax, quantize
pass 2, PSUM drains. ACT: quantize pass 1 (+ weight round). The big x/out
DMAs split across the two HWDGE rings (SP + ACT); the small weight/halo
DMAs are forced after the x loads so they can't steal queue slots.
"""

from contextlib import ExitStack

import numpy as np

import concourse.bacc as bacc
import concourse.bass as bass
import concourse.tile as tile
from concourse import bass_isa, mybir

F32 = mybir.dt.float32
BF16 = mybir.dt.bfloat16
I32 = mybir.dt.int32
MAGIC = float(np.float32(1.5 * 2 ** 23))
R127 = float(np.float32(1.0 / 127.0))
R9216 = float(np.float32(1.0 / 9216.0))

N_CORES = 8
N_IMG = 2           # images per core
FULL_H = FULL_W = 224
C = 32
S = 4               # strips per image
QROWS = 8           # quantize sub-chunk rows


def build_nc(n_img=N_IMG, Hg=FULL_H, Wg=FULL_W, n_cores=N_CORES,
             chunk_rows=None):
    HS = Hg // S
    assert Hg % S == 0 and HS % 2 == 0
    PW = Wg + 2
    NT = 2 * PW
    assert NT <= 512
    BROWS = HS + 3
    BLEN = BROWS * PW
    PR = HS * Wg                     # per-partition strip size in DRAM
    if chunk_rows is None:
        chunk_rows = [24, 24, HS - 48] if HS > 48 else [HS]
    splits = np.cumsum([0] + list(chunk_rows)).tolist()
    assert splits[-1] == HS
    chunks = list(zip(splits[:-1], splits[1:]))
    n_chunks = len(chunks)
    n_tiles = HS // 2
    supers = [(t0, min(4, n_tiles - t0)) for t0 in range(0, n_tiles, 4)]

    nc = bacc.Bacc(
        "TRN2", target_bir_lowering=False, debug=False, num_devices=n_cores
    )
    x_d = nc.dram_tensor("x", [n_img, C, Hg, Wg], F32, kind="ExternalInput").ap()
    w_d = nc.dram_tensor("weight", [32, 32, 3, 3], F32, kind="ExternalInput").ap()
    o_d = nc.dram_tensor("out", [n_img, C, Hg, Wg], F32, kind="ExternalOutput").ap()
    wr = w_d.rearrange("o c dy dx -> c (dy dx) o")
    # (c s)/(o s) merge into a single uniform-stride partition dim: p = 4c+s
    xr = x_d.rearrange("n c (s h) w -> n (c s) h w", s=S)
    orr = o_d.rearrange("n o (s h) w -> n (o s) h w", s=S)
    xsv = x_d.rearrange("n c (s h) w -> n (c s) (h w)", s=S)   # [n, 128, PR]

    with tile.TileContext(nc) as tc, ExitStack() as ctx:
        wp = ctx.enter_context(tc.tile_pool(name="wp", bufs=1))
        xfp = ctx.enter_context(tc.tile_pool(name="xfp", bufs=1))
        xqp = ctx.enter_context(tc.tile_pool(name="xqp", bufs=1))
        qtp = ctx.enter_context(tc.tile_pool(name="qtp", bufs=2))
        psp = ctx.enter_context(tc.tile_pool(name="psp", bufs=8, space="PSUM"))
        stp = ctx.enter_context(tc.tile_pool(name="stp", bufs=3))
        drp = ctx.enter_context(tc.tile_pool(name="drp", bufs=1, space="DRAM"))

        # ---- full-width pure-2D x loads, alternating HWDGE rings ----------
        xf_tiles = {}
        for n in range(n_img):
            for ci, (r0, r1) in enumerate(chunks):
                nr = r1 - r0
                xf_tiles[(n, ci)] = xfp.tile(
                    [128, nr * Wg], F32, name=f"xf_{n}_{ci}", tag=f"xf_{n}_{ci}"
                )
        # ---- warmup collective ---------------------------------------------
        # The first collective in a NEFF pays ~65us of one-time ncfw/ring
        # setup; later ones take ~5-15us. Fire a dummy AllReduce immediately
        # (overlapping the x load) so the real one below runs warm.
        if n_cores > 1:
            ccw_z = wp.tile([1, 1], F32, name="ccw_z")
            nc.vector.memset(ccw_z[:, :], 0.0)
            ccw_in = drp.tile([1, 1], F32, name="ccw_in")
            ccw_out = drp.tile(
                [1, 1], F32, name="ccw_out",
                addr_space="Shared" if n_cores > 4 else "Local",
            )
            nc.scalar.dma_start(ccw_in[:, :], ccw_z[:, :])
            nc.gpsimd.collective_compute(
                "AllReduce", mybir.AluOpType.max,
                replica_groups=[list(range(n_cores))],
                ins=[ccw_in[:, :].opt()], outs=[ccw_out[:, :].opt()],
            )
            ccw_sink = wp.tile([1, 1], F32, name="ccw_sink")
            nc.sync.dma_start(ccw_sink[:, :], ccw_out[:, :])

        last_interior = None
        for ci, (r0, r1) in enumerate(chunks):
            for n in range(n_img):
                eng = nc.sync if n % 2 == 0 else nc.scalar
                last_interior = eng.dma_start(
                    xf_tiles[(n, ci)][:, :],
                    xr[n, :, r0:r1, :],
                )

        # halo tiles: ht[p] = image row above partition p's strip (garbage
        # in p%4==0, zero in p=0); hb[p] = image row below (garbage in
        # p%4==3, zero in p=127). Forced after the big x loads.
        halo_tiles = {}
        first_small = None
        for n in range(n_img):
            ht = wp.tile([128, Wg], F32, name=f"ht_{n}", tag=f"ht_{n}")
            hb = wp.tile([128, Wg], F32, name=f"hb_{n}", tag=f"hb_{n}")
            halo_tiles[n] = (ht, hb)
            nc.vector.memset(ht[0:32, :], 0.0)
            nc.vector.memset(hb[96:128, :], 0.0)
            d = nc.scalar.dma_start(ht[1:128, :], xsv[n, 0:127, PR - Wg:PR])
            first_small = first_small or d
            nc.scalar.dma_start(hb[0:127, :], xsv[n, 1:128, 0:Wg])
        if first_small is not None and last_interior is not None:
            bass._add_dep_helper(
                first_small.ins, last_interior.ins, sync=True,
                reason="keep HW DMA queue slots clear for the big x loads",
            )

        # ---- constants: halo masks, spread matrix, block mask -------------
        iot = wp.tile([128, 1], I32, name="iot")
        nc.gpsimd.iota(iot[:, :], pattern=[[0, 1]], base=0, channel_multiplier=1)
        iand = wp.tile([128, 1], I32, name="iand")
        nc.vector.tensor_scalar(iand[:, :], iot[:, :], 3, None,
                                op0=mybir.AluOpType.bitwise_and)
        mask_t = wp.tile([128, 1], F32, name="mask_t")   # 0 where p%4==0
        nc.vector.tensor_scalar(mask_t[:, :], iand[:, :], 0, None,
                                op0=mybir.AluOpType.not_equal)
        mask_b = wp.tile([128, 1], F32, name="mask_b")   # 0 where p%4==3
        nc.vector.tensor_scalar(mask_b[:, :], iand[:, :], 3, None,
                                op0=mybir.AluOpType.not_equal)
        ones32 = wp.tile([32, 128], BF16, name="ones32")
        nc.vector.memset(ones32[:, :], 1.0)
        asp1 = wp.tile([32, 128], BF16, name="asp1")
        nc.gpsimd.affine_select(
            asp1[:, :], ones32[:, :], pattern=[[1, 128]], base=0,
            channel_multiplier=-4, compare_op=mybir.AluOpType.is_ge, fill=0.0,
        )
        a_sp = wp.tile([32, 128], BF16, name="a_sp")     # A[c, 4c+s] = 1
        nc.gpsimd.affine_select(
            a_sp[:, :], asp1[:, :], pattern=[[-1, 128]], base=3,
            channel_multiplier=4, compare_op=mybir.AluOpType.is_ge, fill=0.0,
        )
        ipm = wp.tile([128, 128], I32, name="ipm")       # p - m
        nc.gpsimd.iota(ipm[:, :], pattern=[[-1, 128]], base=0,
                       channel_multiplier=1)
        ipm2 = wp.tile([128, 128], I32, name="ipm2")
        nc.vector.tensor_scalar(ipm2[:, :], ipm[:, :], 3, None,
                                op0=mybir.AluOpType.bitwise_and)
        maskm = wp.tile([128, 128], F32, name="maskm")   # 1 where p%4==m%4
        nc.vector.tensor_scalar(maskm[:, :], ipm2[:, :], 0, None,
                                op0=mybir.AluOpType.is_equal)

        # ---------------- local absmax, pipelined with the x DMAs ----------
        # Halo rows duplicate interior rows of neighboring strips, so
        # reducing the interior chunks alone covers every element.
        pmax = wp.tile([128, n_img * n_chunks], F32, name="pmax")
        for n in range(n_img):
            for ci in range(n_chunks):
                k = n * n_chunks + ci
                nc.vector.tensor_reduce(
                    pmax[:, k:k + 1], xf_tiles[(n, ci)][:, :],
                    axis=mybir.AxisListType.X,
                    op=mybir.AluOpType.max, apply_absolute_value=True,
                )

        # ---------------- global activation scale ----------------
        # (runs before the weight path so the DVE queue can't head-of-line
        # block the collective input behind the delayed weight DMA)
        amax = wp.tile([128, 1], F32, name="amax")
        nc.vector.tensor_reduce(
            amax[:, :], pmax[:, :], axis=mybir.AxisListType.X,
            op=mybir.AluOpType.max,
        )
        lmax = wp.tile([128, 1], F32, name="lmax")
        nc.gpsimd.partition_all_reduce(
            lmax[:, :], amax[:, :], channels=128,
            reduce_op=bass_isa.ReduceOp.max,
        )
        gx = wp.tile([1, 1], F32, name="gx")
        if n_cores > 1:
            cc_in = drp.tile([1, 1], F32, name="cc_in")
            cc_out = drp.tile(
                [1, 1], F32, name="cc_out",
                addr_space="Shared" if n_cores > 4 else "Local",
            )
            nc.sync.dma_start(cc_in[:, :], lmax[0:1, :])
            nc.gpsimd.collective_compute(
                "AllReduce", mybir.AluOpType.max,
                replica_groups=[list(range(n_cores))],
                ins=[cc_in[:, :].opt()], outs=[cc_out[:, :].opt()],
            )
            nc.sync.dma_start(gx[:, :], cc_out[:, :])
        else:
            nc.vector.tensor_copy(gx[:, :], lmax[0:1, :])
        sx = wp.tile([1, 1], F32, name="sx")
        nc.vector.tensor_scalar_add(sx[:, :], gx[:, :], 1e-5)
        rec = wp.tile([1, 1], F32, name="rec")
        nc.vector.reciprocal(rec[:, :], sx[:, :])
        rsc = wp.tile([1, 1], F32, name="rsc")      # 127/x_scale
        nc.vector.tensor_scalar_mul(rsc[:, :], rec[:, :], 127.0)
        rvec = wp.tile([128, 1], F32, name="rvec")
        nc.gpsimd.partition_broadcast(rvec[:, :], rsc[:, :], channels=128)
        rap = rvec[:, 0:1]
        rap_t = wp.tile([128, 1], F32, name="rap_t")   # halo scales w/ mask
        nc.vector.tensor_mul(rap_t[:, :], rap, mask_t[:, :])
        rap_b = wp.tile([128, 1], F32, name="rap_b")
        nc.vector.tensor_mul(rap_b[:, :], rap, mask_b[:, :])

        # ---------------- weight quantization (off critical path) ----------
        w_sb = wp.tile([32, 9, 32], F32, name="w_sb")
        nc.scalar.dma_start(w_sb[:, :, :], wr[:, :, :])
        wsum = wp.tile([32, 1], F32, name="wsum")
        nc.vector.tensor_reduce(
            wsum[:, :], w_sb[:, :, :], axis=mybir.AxisListType.XY,
            op=mybir.AluOpType.add, apply_absolute_value=True,
        )
        wall = wp.tile([32, 1], F32, name="wall")
        nc.gpsimd.partition_all_reduce(
            wall[:, :], wsum[:, :], channels=32, reduce_op=bass_isa.ReduceOp.add
        )
        sw = wp.tile([32, 1], F32, name="sw")
        nc.vector.tensor_scalar(
            sw[:, :], wall[:, :], R9216, 1e-5,
            op0=mybir.AluOpType.mult, op1=mybir.AluOpType.add,
        )
        rw = wp.tile([32, 1], F32, name="rw")
        nc.vector.reciprocal(rw[:, :], sw[:, :])
        wrnd = wp.tile([32, 288], F32, name="wrnd")
        nc.scalar.activation(
            wrnd[:, :], w_sb.rearrange("c t o -> c (t o)"),
            mybir.ActivationFunctionType.Copy, bias=MAGIC, scale=rw[:, 0:1],
        )
        wq1 = wp.tile([32, 288], F32, name="wq1")
        nc.vector.tensor_scalar(
            wq1[:, :], wrnd[:, :], -MAGIC, 1.0,
            op0=mybir.AluOpType.add, op1=mybir.AluOpType.min,
        )
        wqb = wp.tile([32, 288], BF16, name="wqb")
        nc.vector.tensor_scalar_max(wqb[:, :], wq1[:, :], -1.0)

        # lhsT[4c+s, 128t + 4o + s] = wq[o, c, t], built on the PE:
        # psum[p, m] = sum_c A[c, p] * wq4[c, m] then mod-4 block mask, with
        # wq4[c, 128t+4o+rep] = wq[o, c, t] (columns repeated 4x via strided
        # free-dim copies)
        wq4 = wp.tile([32, 9 * 128], BF16, name="wq4")
        wq4v = wq4.rearrange("c (t o4) -> c t o4", t=9)
        wqbv = wqb.rearrange("c (t o) -> c t o", t=9)
        for rep in range(4):
            nc.vector.tensor_copy(wq4v[:, :, rep::4], wqbv[:, :, :])
        lhsT = wp.tile([128, 9 * 128], BF16, name="lhsT")
        for t in range(9):
            pb = psp.tile([128, 128], F32, name=f"pb_{t}", tag="ps")
            nc.tensor.matmul(pb[:, :], a_sp[:, :],
                             wq4[:, 128 * t:128 * (t + 1)],
                             start=True, stop=True)
            nc.vector.tensor_mul(
                lhsT[:, 128 * t:128 * (t + 1)], pb[:, :], maskm[:, :]
            )

        # output scale C = (x_scale/127) * w_scale (only drains need it)
        c1 = wp.tile([1, 1], F32, name="c1")
        nc.vector.tensor_scalar_mul(c1[:, :], sx[:, :], R127)
        csc = wp.tile([1, 1], F32, name="csc")
        nc.vector.tensor_mul(csc[:, :], c1[:, :], sw[0:1, :])
        cvec = wp.tile([128, 1], F32, name="cvec")
        nc.gpsimd.partition_broadcast(cvec[:, :], csc[:, :], channels=128)
        cap = cvec[:, 0:1]

        # ---------------- quantize x -> padded bf16 buffer ----------------
        # pass 1 (ACT): t = x*(127/s) + MAGIC  (contiguous -> contiguous)
        # pass 2 (DVE): xq = t - MAGIC -> bf16, written with the padded pitch
        xq_tiles = []
        for n in range(n_img):
            xq = xqp.tile([128, BLEN], BF16, name=f"xq_{n}", tag=f"xq_{n}")
            xq_tiles.append(xq)
            xqv = xq.rearrange("p (r w) -> p r w", w=PW)
            # zero pads: cols 0 / W+1 and the slack row
            nc.vector.memset(xqv[:, :, 0:1], 0.0)
            nc.vector.memset(xqv[:, :, PW - 1:PW], 0.0)
            nc.vector.memset(xqv[:, HS + 2, :], 0.0)
            # halo rows -> xq rows 0 / HS+1; masked scale zeroes pad strips
            ht, hb = halo_tiles[n]
            for src, row, scl in [(ht, 0, rap_t), (hb, HS + 1, rap_b)]:
                qt = qtp.tile([128, QROWS * Wg], F32, name="qt", tag="qt")
                nc.scalar.activation(
                    qt[:, 0:Wg], src[:, :],
                    mybir.ActivationFunctionType.Copy, bias=MAGIC,
                    scale=scl[:, 0:1],
                )
                nc.vector.tensor_scalar_add(
                    xqv[:, row, 1:1 + Wg], qt[:, 0:Wg], -MAGIC
                )
            for ci, (r0, r1) in enumerate(chunks):
                ct = xf_tiles[(n, ci)]
                nr = r1 - r0
                for q0 in range(0, nr, QROWS):
                    q1 = min(q0 + QROWS, nr)
                    qt = qtp.tile([128, QROWS * Wg], F32, name="qt", tag="qt")
                    nq = (q1 - q0) * Wg
                    nc.scalar.activation(
                        qt[:, 0:nq], ct[:, q0 * Wg:q1 * Wg],
                        mybir.ActivationFunctionType.Copy, bias=MAGIC,
                        scale=rap,
                    )
                    nc.vector.tensor_scalar_add(
                        xqv[:, 1 + r0 + q0:1 + r0 + q1, 1:1 + Wg],
                        qt[:, 0:nq].rearrange("p (r w) -> p r w", w=Wg),
                        -MAGIC,
                    )

        # ---------------- conv matmuls + drain + store ----------------
        for n in range(n_img):
            xq = xq_tiles[n]
            for (t0, nb) in supers:
                pst = [
                    psp.tile([128, NT], F32, name=f"ps_{n}_{t0}_{b}", tag="ps")
                    for b in range(nb)
                ]
                for t in range(9):
                    dy, dx = divmod(t, 3)
                    lt = lhsT[:, 128 * t:128 * (t + 1)]
                    for b in range(nb):
                        st = 2 * PW * (t0 + b) + PW * dy + dx
                        nc.tensor.matmul(
                            pst[b][:, :], lt, xq[:, st:st + NT],
                            start=(t == 0), stop=(t == 8),
                        )
                # drain: strided PSUM read (skip pad cols) -> contiguous stage
                stg = stp.tile([128, 8 * Wg], F32, name="stg", tag="stg")
                for b in range(nb):
                    nc.vector.tensor_scalar_mul(
                        stg[:, 2 * b * Wg:2 * (b + 1) * Wg]
                        .rearrange("p (r w) -> p r w", w=Wg),
                        pst[b].rearrange("p (r w) -> p r w", w=PW)[:, :, 0:Wg],
                        cap,
                    )
                nc.sync.dma_start(
                    orr[n, :, 2 * t0:2 * (t0 + nb), :],
                    stg[:, 0:2 * nb * Wg],
                )

    nc.compile()
    return nc


_NC = None


def _get_nc():
    global _NC
    if _NC is None:
        _NC = build_nc()
    return _NC


def run_sharded(x, weight, **spmd_kwargs):
    """Run the SPMD kernel; returns (out, BassKernelResults)."""
    from concourse.bass_utils import run_bass_kernel_spmd

    x = np.ascontiguousarray(x, dtype=np.float32)
    weight = np.ascontiguousarray(weight, dtype=np.float32)
    assert x.shape == (N_CORES * N_IMG, C, FULL_H, FULL_W)
    nc = _get_nc()
    in_maps = [
        {"x": x[c * N_IMG:(c + 1) * N_IMG], "weight": weight}
        for c in range(N_CORES)
    ]
    try:
        res = run_bass_kernel_spmd(nc, in_maps, list(range(N_CORES)),
                                   **spmd_kwargs)
    except Exception:
        # one retry: transient NRT_EXEC_UNIT_UNRECOVERABLE has been observed
        # on a freshly-reset device
        res = run_bass_kernel_spmd(nc, in_maps, list(range(N_CORES)),
                                   **spmd_kwargs)
    out = np.concatenate([res.results[c]["out"] for c in range(N_CORES)], axis=0)
    return out, res


def kernel(x, weight):
    out, _ = run_sharded(x, weight)
    return out

